# revision 1
# baseline (speedup 1.0000x reference)
"""Trainium2 Bass kernel for nn_ExpModel_77824807403811.

The reference simulates a 25-qubit statevector (2^25 fp32 amplitudes) through
8 layers of per-wire RY rotations followed by a nearest-neighbour CNOT chain,
then measures <Z_0> on qubit 0 (the most-significant axis).

Exact algebraic reduction: the backward light cone of Z_0 grows by at most
one wire per layer (conjugating through the CNOT chain CNOT(w,w+1), applied
w=0..NQ-2 in order, spreads Pauli support upward by exactly one wire per
layer; Z on a control commutes).  After 8 layers the Heisenberg operator
U^dag Z_0 U is supported on wires 0..7 only, so <Z_0> equals the same
circuit truncated to the first NQT=8 qubits — all gates on wires >= 8 drop
out exactly.  Verified numerically: the fp64 truncated value is identical
for NQT = 8, 9, 10 (+0.254275475953...) and the full 25-qubit fp32
reference matches it to 3.2e-7.

Device layout (per NeuronCore): state[256] as an SBUF tile [128 partitions x
2 free]; partition index = qubits 0..6 (q0 = MSB), free column = b7.

Per layer L the circuit is RY(q0..q7) then CNOT(0,1)..(6,7).  Mapping:
  - RY(q0..q6) + CNOT(0,1)..(5,6): one 128x128 PE matmul with a
    host-precomputed orthogonal operator W_L (gate operators, not state).
  - CNOT(6,7) (control = partition LSB, target = free bit) followed by the
    NEXT layer's RY(q7) fuse into 4 DVE ops: conjugating RY(q7) through the
    parity-conditional column swap gives per-partition coefficient columns
    alpha/beta/gamma/delta (host-precomputed), so
      T'[:,0] = alpha.PS0 + beta.PS1,  T'[:,1] = gamma.PS0 + delta.PS1
    read straight out of PSUM.
  - The FIRST RY(q7) acts on |0..0>, so the initial state is just three
    memsets (cos/sin baked as immediates).
  - The LAST CNOT(6,7) permutes amplitudes within each partition row, which
    leaves the readout sum of squares per partition invariant — skipped.
Readout: each PSUM column is squared in place by using itself as the
per-partition scalar operand (scalar reads are exempt from the
one-PSUM-operand rule, so no PSUM->SBUF copy), summed pairwise; the [128,1]
partial-sum vector is DMA'd out and the final +/-1(qubit-0)-signed
128-element reduction happens in the host-side gather (in fp64, which is
also numerically tighter).

The same tiny program is replicated SPMD on all 8 cores (the light-cone
reduction removes any need for cross-core exchange); core 0's partials are
used.
"""

import numpy as np

NQ = 25
DEPTH = 8
NQT = 8          # truncated (light-cone) qubit count — exact for DEPTH=8
NPART_Q = 7      # qubits 0..6 live on the partition axis
P = 128
F = 2            # free axis: qubit 7
N_CORES = 8
NCONST = 4 * (DEPTH - 1)       # fused coeff cols per boundary


def _host_layer_mats(theta):
    """Per-layer 128x128 operator for RY(q0..q6) then CNOT(0,1)..(5,6),
    returned as the matmul's stationary operand lhsT (lhsT[k, m] = W[m, k]),
    concatenated along the free axis -> [128, DEPTH*128] fp32."""
    th = np.asarray(theta, np.float64)
    mats = []
    for layer in range(DEPTH):
        c = np.cos(th[layer] / 2.0)
        s = np.sin(th[layer] / 2.0)
        W = np.array([[1.0]])
        for w in range(NPART_Q):
            R = np.array([[c[w], -s[w]], [s[w], c[w]]])
            W = np.kron(W, R)          # q0 outermost -> partition MSB
        # CNOT(w, w+1) for w = 0..5, in circuit order, as row swaps of W
        for w in range(NPART_Q - 1):
            Wv = W.reshape((2,) * NPART_Q + (P,))
            sl0 = [slice(None)] * (NPART_Q + 1)
            sl1 = [slice(None)] * (NPART_Q + 1)
            sl0[w], sl0[w + 1] = 1, 0
            sl1[w], sl1[w + 1] = 1, 1
            tmp = Wv[tuple(sl0)].copy()
            Wv[tuple(sl0)] = Wv[tuple(sl1)]
            Wv[tuple(sl1)] = tmp
            W = Wv.reshape(P, P)
        mats.append(np.ascontiguousarray(W.T).astype(np.float32))
    return np.concatenate(mats, axis=1)


def _host_consts(theta):
    """[128, NCONST] fp32: per-boundary fused CNOT(6,7)+RY(q7) coefficient
    columns (alpha, beta, gamma, delta for boundaries L=0..6, using layer
    L+1's q7 angle)."""
    th = np.asarray(theta, np.float64)
    pidx = np.arange(P)
    even = (pidx % 2 == 0)          # q6 = 0
    C = np.zeros((P, NCONST), np.float64)
    for L in range(DEPTH - 1):
        c7 = np.cos(th[L + 1, 7] / 2.0)
        s7 = np.sin(th[L + 1, 7] / 2.0)
        C[:, 4 * L + 0] = np.where(even, c7, -s7)    # alpha
        C[:, 4 * L + 1] = np.where(even, -s7, c7)    # beta
        C[:, 4 * L + 2] = np.where(even, s7, c7)     # gamma
        C[:, 4 * L + 3] = np.where(even, c7, s7)     # delta
    return C.astype(np.float32)


def _emit(nc, w_ap, c_ap, out_ap, theta):
    import concourse.mybir as mybir
    from concourse.tile import TileContext

    f32 = mybir.dt.float32
    mult = mybir.AluOpType.mult
    add = mybir.AluOpType.add

    c70 = float(np.cos(np.float64(theta[0, 7]) / 2.0))
    s70 = float(np.sin(np.float64(theta[0, 7]) / 2.0))

    with TileContext(nc) as tc:
        with tc.tile_pool(name="pool", bufs=1) as pool, \
             tc.tile_pool(name="tpool", bufs=2) as tpool, \
             tc.tile_pool(name="psum", bufs=2, space="PSUM") as psum:
            Ws = [pool.tile([P, P], f32, tag=f"W{L}", name=f"W{L}")
                  for L in range(DEPTH)]
            C = pool.tile([P, NCONST], f32, tag="C")
            SS = pool.tile([P, 1], f32, tag="SS")
            SQ = pool.tile([P, F], f32, tag="SQ")

            # Every DMA occupies its issuing engine's queue ~500ns, so
            # round-robin the per-layer W chunks over four different engine
            # queues; only chunk 0 then gates the first matmul and every
            # later chunk lands before its layer needs it.
            # SP has the lowest DMA init latency -> W0 first there; the
            # rest are placed so every chunk lands before its layer, with
            # the consts DMA pinned to the head of the ACT queue.
            queue_of = {0: nc.sync, 2: nc.sync, 5: nc.sync, 7: nc.sync,
                        3: nc.scalar, 6: nc.scalar,
                        1: nc.gpsimd, 4: nc.gpsimd}
            with tc.high_priority():
                nc.sync.dma_start(Ws[0][:], w_ap[:, 0:P])
                nc.scalar.dma_start(C[:], c_ap)
                nc.gpsimd.dma_start(Ws[1][:], w_ap[:, P:2 * P])
            for L in range(2, DEPTH):
                queue_of[L].dma_start(Ws[L][:], w_ap[:, L * P:(L + 1) * P])

            # initial state after RY(q7) of layer 0 on |0..0>
            T = tpool.tile([P, F], f32, tag="T")
            nc.vector.memset(T[:], 0.0)
            nc.vector.memset(T[0:1, 0:1], c70)
            nc.vector.memset(T[0:1, 1:2], s70)

            for L in range(DEPTH):
                PS = psum.tile([P, F], f32, tag="PS")
                nc.tensor.matmul(PS[:], Ws[L][:], T[:],
                                 start=True, stop=True)
                if L < DEPTH - 1:
                    # fused CNOT(6,7) + RY(q7) of layer L+1, from PSUM
                    al = C[:, 4 * L + 0:4 * L + 1]
                    be = C[:, 4 * L + 1:4 * L + 2]
                    ga = C[:, 4 * L + 2:4 * L + 3]
                    de = C[:, 4 * L + 3:4 * L + 4]
                    T = tpool.tile([P, F], f32, tag="T")
                    U = tpool.tile([P, F], f32, tag="U")
                    nc.vector.tensor_scalar_mul(U[:, 0:1], PS[:, 0:1], al)
                    nc.vector.scalar_tensor_tensor(
                        T[:, 0:1], PS[:, 1:2], be, U[:, 0:1], mult, add)
                    nc.vector.tensor_scalar_mul(U[:, 1:2], PS[:, 0:1], ga)
                    nc.vector.scalar_tensor_tensor(
                        T[:, 1:2], PS[:, 1:2], de, U[:, 1:2], mult, add)
                else:
                    # readout: per-partition sum of squares (the final
                    # CNOT(6,7) permutes within rows — sum is invariant).
                    # Square each column using itself as the per-partition
                    # scalar operand (scalar reads are exempt from the
                    # one-PSUM-operand rule), skipping the PSUM->SBUF copy.
                    # DVE only: ScalarE activation would pull in a ~1.4us
                    # LoadActFuncSet that blocks the ACT DMA queue head.
                    nc.vector.tensor_scalar_mul(
                        SQ[:, 0:1], PS[:, 0:1], PS[:, 0:1])
                    nc.vector.tensor_scalar_mul(
                        SQ[:, 1:2], PS[:, 1:2], PS[:, 1:2])
                    nc.vector.tensor_add(SS[:], SQ[:, 0:1], SQ[:, 1:2])

            # ship the per-partition sums; the signed 128-element
            # reduction is part of the host-side gather
            nc.sync.dma_start(out_ap, SS[:])
    return nc


def _build(theta):
    import concourse.bacc as bacc
    import concourse.mybir as mybir

    f32 = mybir.dt.float32
    nc = bacc.Bacc("TRN2", target_bir_lowering=False, debug=False)
    w_d = nc.dram_tensor("w", [P, DEPTH * P], f32, kind="ExternalInput")
    c_d = nc.dram_tensor("c", [P, NCONST], f32, kind="ExternalInput")
    out_d = nc.dram_tensor("out", [P, 1], f32, kind="ExternalOutput")
    _emit(nc, w_d.ap(), c_d.ap(), out_d.ap(), theta)
    nc.finalize()
    return nc


_NC_CACHE = {}


def kernel(theta, _trace=False, _return_results=False):
    theta = np.asarray(theta)
    assert theta.shape == (DEPTH, NQ), theta.shape
    from concourse.bass_utils import run_bass_kernel_spmd

    key = theta.tobytes()
    if _NC_CACHE.get("key") != key:
        _NC_CACHE["nc"] = _build(theta)
        _NC_CACHE["key"] = key
    nc = _NC_CACHE["nc"]

    in_map = {"w": _host_layer_mats(theta), "c": _host_consts(theta)}
    res = run_bass_kernel_spmd(
        nc,
        in_maps=[in_map] * N_CORES,
        core_ids=list(range(N_CORES)),
        trace=_trace,
    )
    ss = res.results[0]["out"][:, 0].astype(np.float64)
    sign = np.where(np.arange(P) < 64, 1.0, -1.0)
    out = np.array(np.float32(np.dot(sign, ss)), dtype=np.float32)
    if _return_results:
        return out, res
    return out

